# revision 14
# baseline (speedup 1.0000x reference)
"""DMTetGeometry kernel for trn2 (8 NeuronCores, SPMD).

Strategy (data-parallel, per sharding hint):
- Edges (5,990,000 unique) row-sharded contiguously across 8 cores;
  tets (1,000,000) row-sharded; pos/sdf replicated.
- On device, per core: pack (pos,sdf) into a 16B/vertex record table in
  DRAM, then per 8192-edge chunk indirect-DMA-gather both endpoint
  records (128 offsets per instruction, one per partition), compute the
  crossing/interpolation math on DVE, and write verts.
- Tets: indirect-gather sdf at the 4 corners, build occupancy masks,
  evaluate the 16-entry triangle table with a select tree (bits of the
  tet occupancy code), mask invalid faces/uv_idx to -1.
- uvs is a closed-form grid; generated on device from tiny per-core
  constant rows/columns.
Outputs are unpadded + concatenated on the host.
"""
import numpy as np
from contextlib import ExitStack
from dataclasses import dataclass

import concourse.bass as bass
import concourse.bacc as bacc
import concourse.mybir as mybir
import concourse.tile as tile
from concourse.bass import AP
from concourse.bass_utils import run_bass_kernel_spmd

F32 = mybir.dt.float32
I32 = mybir.dt.int32
P = 128

TRI_TABLE = np.array([
    [-1, -1, -1, -1, -1, -1], [1, 0, 2, -1, -1, -1], [4, 0, 3, -1, -1, -1],
    [1, 4, 2, 1, 3, 4], [3, 1, 5, -1, -1, -1], [2, 3, 0, 2, 5, 3],
    [1, 4, 0, 1, 5, 4], [4, 2, 5, -1, -1, -1], [4, 5, 2, -1, -1, -1],
    [4, 1, 0, 4, 5, 1], [3, 2, 0, 3, 5, 2], [1, 3, 5, -1, -1, -1],
    [4, 1, 2, 4, 3, 1], [3, 0, 4, -1, -1, -1], [2, 0, 1, -1, -1, -1],
    [-1, -1, -1, -1, -1, -1]], dtype=np.int32)
TRI_CLIP = np.clip(TRI_TABLE, 0, 5)


@dataclass
class Cfg:
    n_cores: int = 8
    nv_pad: int = 600064        # vertices padded to 128*F
    e_f: int = 64               # edge free-dim per chunk
    ne_chunks: int = 92         # per-core edge chunks
    t_f: int = 64
    nt_chunks: int = 16
    uv_cols: int = 8000         # Ngrid*8
    uv_rows: int = 125          # i-rows per core
    rf: int = 586               # rec-pack free dim (NV = 128*rf*n_pack)

    @property
    def e_chunk(self):
        return P * self.e_f

    @property
    def e_pad(self):
        return self.e_chunk * self.ne_chunks

    @property
    def t_chunk(self):
        return P * self.t_f

    @property
    def t_pad(self):
        return self.t_chunk * self.nt_chunks


def build_nc(cfg: Cfg):
    nc = bacc.Bacc(None, target_bir_lowering=False)
    NV = cfg.nv_pad
    pos = nc.dram_tensor("pos", [NV, 3], F32, kind="ExternalInput")
    sdf = nc.dram_tensor("sdf", [NV, 1], F32, kind="ExternalInput")
    ue = nc.dram_tensor("ue", [cfg.e_pad, 2], I32, kind="ExternalInput")
    tet = nc.dram_tensor("tet", [cfg.t_pad, 4], I32, kind="ExternalInput")
    imap = nc.dram_tensor("imap", [cfg.t_pad, 6], I32, kind="ExternalInput")
    toff = nc.dram_tensor("toff", [P, 1], F32, kind="ExternalInput")
    xpy = nc.dram_tensor("xpy", [P, cfg.uv_cols], F32, kind="ExternalInput")
    yind = nc.dram_tensor("yind", [P, cfg.uv_cols], F32, kind="ExternalInput")
    ycol = nc.dram_tensor("ycol", [P, 1], F32, kind="ExternalInput")

    verts_o = nc.dram_tensor("verts_o", [cfg.e_pad, 3], F32,
                             kind="ExternalOutput")
    faces_o = nc.dram_tensor("faces_o", [cfg.t_pad, 6], I32,
                             kind="ExternalOutput")
    uvidx_o = nc.dram_tensor("uvidx_o", [cfg.t_pad, 6], I32,
                             kind="ExternalOutput")
    uvs_o = nc.dram_tensor("uvs_o", [cfg.uv_rows, cfg.uv_cols], F32,
                           kind="ExternalOutput")
    dbg_o = nc.dram_tensor("dbg_o", [5, P, cfg.e_f], F32,
                           kind="ExternalOutput")

    rec = nc.dram_tensor("rec", [NV, 4], F32)  # packed (pos.xyz, sdf)

    EF, TF = cfg.e_f, cfg.t_f

    with tile.TileContext(nc) as tc, ExitStack() as ctx:
        sb = ctx.enter_context(tc.tile_pool(name="sb", bufs=3))
        sbc = ctx.enter_context(tc.tile_pool(name="sbc", bufs=1))

        # ---- pack rec = [pos | sdf], sequential streaming ----
        RF = cfg.rf
        n_pack = NV // (P * RF)
        assert NV == n_pack * P * RF
        for i in range(n_pack):
            v0 = i * P * RF
            pt = sb.tile([P, RF, 3], F32, tag="pt", bufs=2)
            st = sb.tile([P, RF], F32, tag="st", bufs=2)
            rt = sb.tile([P, RF, 4], F32, tag="rt", bufs=2)
            nc.sync.dma_start(
                out=pt[:], in_=pos[v0:v0 + P * RF, :].rearrange(
                    "(p f) c -> p f c", p=P))
            nc.sync.dma_start(
                out=st[:], in_=sdf[v0:v0 + P * RF, :].rearrange(
                    "(p f) one -> p (f one)", p=P))
            nc.vector.tensor_copy(rt[:, :, 0:3], pt[:])
            nc.vector.tensor_copy(rt[:, :, 3], st[:])
            nc.sync.dma_start(
                out=rec[v0:v0 + P * RF, :].rearrange(
                    "(p f) c -> p f c", p=P), in_=rt[:])

        # ---- uvs (pure grid) ----
        xt = sbc.tile([P, cfg.uv_cols], F32)
        yt = sbc.tile([P, cfg.uv_cols], F32)
        yc = sbc.tile([P, 1], F32)
        nc.sync.dma_start(out=xt[:], in_=xpy[:])
        nc.sync.dma_start(out=yt[:], in_=yind[:])
        nc.sync.dma_start(out=yc[:], in_=ycol[:])
        uvt = sb.tile([P, cfg.uv_cols], F32, tag="uvt", bufs=1)
        nc.vector.tensor_tensor(
            uvt[:], yt[:], yc[:].to_broadcast([P, cfg.uv_cols]),
            mybir.AluOpType.mult)
        nc.vector.tensor_tensor(uvt[:], uvt[:], xt[:],
                                mybir.AluOpType.add)
        nc.sync.dma_start(out=uvs_o[:, :], in_=uvt[:cfg.uv_rows, :])

        # ---- constants ----
        one = sbc.tile([P, 1], F32, tag="one")
        nc.vector.memset(one[:], 1.0)
        negone = sbc.tile([P, 1], F32, tag="negone")
        nc.vector.memset(negone[:], -1.0)
        tof = sbc.tile([P, 1], F32, tag="tof")
        nc.sync.dma_start(out=tof[:], in_=toff[:])

        # ---- edge interpolation ----
        for ch in range(cfg.ne_chunks):
            e0 = ch * cfg.e_chunk
            esl = ue[e0:e0 + cfg.e_chunk, :].rearrange(
                "(p f) c -> p f c", p=P)  # [P, EF, 2]
            it = sb.tile([P, EF, 2], I32, tag="it")
            nc.sync.dma_start(out=it[:], in_=esl)
            g0 = sb.tile([P, EF, 4], F32, tag="g0")
            g1 = sb.tile([P, EF, 4], F32, tag="g1")
            for f in range(EF):
                nc.gpsimd.indirect_dma_start(
                    out=g0[:, f, :], out_offset=None, in_=rec[:],
                    in_offset=bass.IndirectOffsetOnAxis(
                        ap=it[:, f, 0:1], axis=0))
                nc.gpsimd.indirect_dma_start(
                    out=g1[:, f, :], out_offset=None, in_=rec[:],
                    in_offset=bass.IndirectOffsetOnAxis(
                        ap=it[:, f, 1:2], axis=0))
            s0 = g0[:, :, 3]
            s1 = g1[:, :, 3]
            occ0 = sb.tile([P, EF], F32, tag="occ0")
            occ1 = sb.tile([P, EF], F32, tag="occ1")
            crs = sb.tile([P, EF], F32, tag="crs")
            den = sb.tile([P, EF], F32, tag="den")
            w0 = sb.tile([P, EF], F32, tag="w0")
            w1 = sb.tile([P, EF], F32, tag="w1")
            nc.vector.tensor_scalar(occ0[:], s0, 0.0, None,
                                    mybir.AluOpType.is_gt)
            nc.vector.tensor_scalar(occ1[:], s1, 0.0, None,
                                    mybir.AluOpType.is_gt)
            nc.vector.tensor_tensor(crs[:], occ0[:], occ1[:],
                                    mybir.AluOpType.not_equal)
            nc.vector.tensor_tensor(den[:], s0, s1,
                                    mybir.AluOpType.subtract)
            crsi = sb.tile([P, EF], I32, tag="crsi")
            nc.vector.tensor_copy(crsi[:], crs[:])
            dsafe = sb.tile([P, EF], F32, tag="dsafe")
            nc.vector.select(dsafe[:], crsi[:], den[:],
                             one[:].to_broadcast([P, EF]))
            # w1 = s0/den ; w0 = -s1/den  (via reciprocal)
            rcp = sb.tile([P, EF], F32, tag="rcp")
            nc.vector.reciprocal(rcp[:], dsafe[:])
            nc.vector.tensor_tensor(w1[:], s0, rcp[:],
                                    mybir.AluOpType.mult)
            nc.vector.tensor_tensor(w0[:], s1, rcp[:],
                                    mybir.AluOpType.mult)
            nc.vector.tensor_scalar(w0[:], w0[:], -1.0, None,
                                    mybir.AluOpType.mult)
            vt = sb.tile([P, EF, 3], F32, tag="vt")
            tmp = sb.tile([P, EF], F32, tag="tmp")
            for c in range(3):
                nc.vector.tensor_tensor(tmp[:], g0[:, :, c], w0[:],
                                        mybir.AluOpType.mult)
                nc.vector.tensor_tensor(vt[:, :, c], g1[:, :, c], w1[:],
                                        mybir.AluOpType.mult)
                nc.vector.tensor_tensor(vt[:, :, c], vt[:, :, c], tmp[:],
                                        mybir.AluOpType.add)
                nc.vector.tensor_tensor(vt[:, :, c], vt[:, :, c], crs[:],
                                        mybir.AluOpType.mult)
            nc.sync.dma_start(
                out=verts_o[e0:e0 + cfg.e_chunk, :].rearrange(
                    "(p f) c -> p f c", p=P), in_=vt[:])
            if ch == 0:
                nc.sync.dma_start(out=dbg_o[0], in_=g0[:, :, 3])
                nc.sync.dma_start(out=dbg_o[1], in_=g1[:, :, 3])
                nc.sync.dma_start(out=dbg_o[2], in_=w0[:])
                nc.sync.dma_start(out=dbg_o[3], in_=w1[:])
                nc.sync.dma_start(out=dbg_o[4], in_=crs[:])

        # ---- tets: occupancy + tri table + uv_idx ----
        for ch in range(cfg.nt_chunks):
            t0 = ch * cfg.t_chunk
            tt = sb.tile([P, TF, 4], I32, tag="tt")
            nc.sync.dma_start(out=tt[:], in_=tet[t0:t0 + cfg.t_chunk, :]
                              .rearrange("(p f) c -> p f c", p=P))
            im = sb.tile([P, TF, 6], I32, tag="im")
            nc.sync.dma_start(out=im[:], in_=imap[t0:t0 + cfg.t_chunk, :]
                              .rearrange("(p f) c -> p f c", p=P))
            imf = sb.tile([P, TF, 6], F32, tag="imf")
            nc.vector.tensor_copy(imf[:], im[:])
            sg = sb.tile([P, TF, 4], F32, tag="sg")
            for f in range(TF):
                for k in range(4):
                    nc.gpsimd.indirect_dma_start(
                        out=sg[:, f, k:k + 1], out_offset=None, in_=sdf[:],
                        in_offset=bass.IndirectOffsetOnAxis(
                            ap=tt[:, f, k:k + 1], axis=0))
            occ = [sb.tile([P, TF], F32, tag=f"ob{k}", name=f"ob{k}") for k in range(4)]
            for k in range(4):
                nc.vector.tensor_scalar(occ[k][:], sg[:, :, k], 0.0, None,
                                        mybir.AluOpType.is_gt)
            ssum = sb.tile([P, TF], F32, tag="ssum")
            nc.vector.tensor_tensor(ssum[:], occ[0][:], occ[1][:],
                                    mybir.AluOpType.add)
            nc.vector.tensor_tensor(ssum[:], ssum[:], occ[2][:],
                                    mybir.AluOpType.add)
            nc.vector.tensor_tensor(ssum[:], ssum[:], occ[3][:],
                                    mybir.AluOpType.add)
            ntri = sb.tile([P, TF], F32, tag="ntri")
            # ntri = min(ssum, 4-ssum, 2)
            nc.vector.tensor_scalar(ntri[:], ssum[:], -1.0, 4.0,
                                    mybir.AluOpType.mult,
                                    mybir.AluOpType.add)
            nc.vector.tensor_tensor(ntri[:], ntri[:], ssum[:],
                                    mybir.AluOpType.min)
            nc.vector.tensor_scalar_min(ntri[:], ntri[:], 2.0)
            tv = [sb.tile([P, TF], I32, tag=f"tv{i}", name=f"tv{i}") for i in range(2)]
            nc.vector.tensor_scalar(tv[0][:], ntri[:], 0.0, None,
                                    mybir.AluOpType.is_gt)
            nc.vector.tensor_scalar(tv[1][:], ntri[:], 1.0, None,
                                    mybir.AluOpType.is_gt)
            occi = [sb.tile([P, TF], I32, tag=f"oci{k}", name=f"oci{k}")
                    for k in range(4)]
            for k in range(4):
                nc.vector.tensor_copy(occi[k][:], occ[k][:])
            fc = sb.tile([P, TF, 6], F32, tag="fc")
            t8 = [sb.tile([P, TF], F32, tag=f"t8{i}", name=f"t8{i}", bufs=2) for i in range(8)]
            for j in range(6):
                # select tree over tetindex bits: leaf k = imf[:,:,TRI_CLIP[k,j]]
                for k2 in range(8):
                    nc.vector.select(t8[k2][:], occi[0][:],
                                     imf[:, :, int(TRI_CLIP[2 * k2 + 1, j])],
                                     imf[:, :, int(TRI_CLIP[2 * k2, j])])
                for k2 in range(4):
                    nc.vector.select(t8[k2][:], occi[1][:],
                                     t8[2 * k2 + 1][:], t8[2 * k2][:])
                for k2 in range(2):
                    nc.vector.select(t8[k2][:], occi[2][:],
                                     t8[2 * k2 + 1][:], t8[2 * k2][:])
                nc.vector.select(t8[0][:], occi[3][:], t8[1][:], t8[0][:])
                nc.vector.select(fc[:, :, j], tv[j // 3][:], t8[0][:],
                                 negone[:].to_broadcast([P, TF]))
            fci = sb.tile([P, TF, 6], I32, tag="fci")
            nc.vector.tensor_copy(fci[:], fc[:])
            nc.sync.dma_start(
                out=faces_o[t0:t0 + cfg.t_chunk, :].rearrange(
                    "(p f) c -> p f c", p=P), in_=fci[:])
            # uv_idx: base = 4*t_global ; cols = [b, b+1, b+2, b, b+2, b+3]
            ti = sb.tile([P, TF], I32, tag="ti")
            nc.gpsimd.iota(ti[:], [[1, TF]], base=t0, channel_multiplier=TF)
            tg = sb.tile([P, TF], F32, tag="tg")
            nc.vector.tensor_copy(tg[:], ti[:])
            nc.vector.tensor_tensor(tg[:], tg[:], tof[:].to_broadcast([P, TF]),
                                    mybir.AluOpType.add)
            nc.vector.tensor_scalar(tg[:], tg[:], 4.0, None,
                                    mybir.AluOpType.mult)
            uvx = sb.tile([P, TF, 6], F32, tag="uvx")
            uvo = sb.tile([P, TF, 6], F32, tag="uvo")
            adds = [0.0, 1.0, 2.0, 0.0, 2.0, 3.0]
            for j in range(6):
                nc.vector.tensor_scalar(uvx[:, :, j], tg[:], adds[j], None,
                                        mybir.AluOpType.add)
                nc.vector.select(uvo[:, :, j], tv[j // 3][:], uvx[:, :, j],
                                 negone[:].to_broadcast([P, TF]))
            uvi = sb.tile([P, TF, 6], I32, tag="uvi")
            nc.vector.tensor_copy(uvi[:], uvo[:])
            nc.sync.dma_start(
                out=uvidx_o[t0:t0 + cfg.t_chunk, :].rearrange(
                    "(p f) c -> p f c", p=P), in_=uvi[:])

    nc.compile()
    return nc


# ---------------- host side ----------------

_CACHE = {}


def _get_nc(cfg: Cfg):
    key = tuple(sorted(cfg.__dict__.items()))
    if key not in _CACHE:
        _CACHE[key] = build_nc(cfg)
    return _CACHE[key]


def make_in_maps(cfg: Cfg, pos_nx3, sdf_n, tet_fx4, unique_edges, idx_map,
                 n_verts, n_tets, n_edges, ngrid):
    pos_nx3 = np.asarray(pos_nx3, np.float32)
    sdf_n = np.asarray(sdf_n, np.float32)
    tet_fx4 = np.asarray(tet_fx4, np.int32)
    unique_edges = np.asarray(unique_edges, np.int32)
    idx_map = np.asarray(idx_map, np.int32)
    C = cfg.n_cores
    NV = cfg.nv_pad
    pos_p = np.zeros((NV, 3), np.float32)
    pos_p[:n_verts] = pos_nx3
    sdf_p = np.zeros((NV, 1), np.float32)
    sdf_p[:n_verts, 0] = sdf_n

    e_sh = -(-n_edges // C)
    t_sh = -(-n_tets // C)
    assert e_sh <= cfg.e_pad and t_sh <= cfg.t_pad

    pad = 0.9 / ngrid
    px = np.array([0.0, pad, pad, 0.0], np.float32)
    py = np.array([0.0, 0.0, pad, pad], np.float32)
    lin = (np.arange(ngrid, dtype=np.float32)) / ngrid
    xpy = np.zeros((1, cfg.uv_cols), np.float32)
    yind = np.zeros((1, cfg.uv_cols), np.float32)
    # replicated to all partitions after fill (see below)
    for j in range(ngrid):
        for q in range(4):
            b = (j * 4 + q) * 2
            xpy[0, b] = lin[j] + px[q]
            xpy[0, b + 1] = py[q]
            yind[0, b + 1] = 1.0

    xpy = np.broadcast_to(xpy, (P, cfg.uv_cols)).copy()
    yind = np.broadcast_to(yind, (P, cfg.uv_cols)).copy()
    in_maps = []
    for c in range(C):
        ue_c = unique_edges[c * e_sh:(c + 1) * e_sh]
        tet_c = tet_fx4[c * t_sh:(c + 1) * t_sh]
        im_c = idx_map[c * t_sh:(c + 1) * t_sh]
        uec = np.zeros((cfg.e_pad, 2), np.int32)
        uec[:len(ue_c)] = ue_c
        tetc = np.zeros((cfg.t_pad, 4), np.int32)
        tetc[:len(tet_c)] = tet_c
        imc = np.zeros((cfg.t_pad, 6), np.int32)
        imc[:len(im_c)] = im_c
        toff = np.full((P, 1), np.float32(c * t_sh), np.float32)
        ycol = ((np.arange(P, dtype=np.float32) + c * cfg.uv_rows) / ngrid
                ).reshape(P, 1).astype(np.float32)
        in_maps.append({
            "pos": pos_p, "sdf": sdf_p, "ue": uec, "tet": tetc,
            "imap": imc, "toff": toff, "xpy": xpy, "yind": yind,
            "ycol": ycol,
        })
    return in_maps


def assemble(cfg: Cfg, results, n_verts, n_tets, n_edges, ngrid):
    C = cfg.n_cores
    e_sh = -(-n_edges // C)
    t_sh = -(-n_tets // C)
    verts = np.concatenate(
        [results[c]["verts_o"][:e_sh] for c in range(C)],
        axis=0)[:n_edges]
    faces = np.concatenate(
        [results[c]["faces_o"][:t_sh] for c in range(C)],
        axis=0)[:n_tets].reshape(n_tets, 2, 3)
    uv_idx = np.concatenate(
        [results[c]["uvidx_o"][:t_sh] for c in range(C)],
        axis=0)[:n_tets].reshape(n_tets, 2, 3)
    uvs = np.concatenate(
        [results[c]["uvs_o"] for c in range(C)], axis=0)[:ngrid]
    uvs = uvs.reshape(ngrid * ngrid * 4, 2)
    return verts, faces, uvs, uv_idx


def kernel(pos_nx3, sdf_n, tet_fx4, unique_edges, idx_map):
    n_verts = pos_nx3.shape[0]
    n_tets = tet_fx4.shape[0]
    n_edges = unique_edges.shape[0]
    ngrid = int(np.ceil(np.sqrt((2 * n_tets + 1) // 2)))
    cfg = Cfg()
    nc = _get_nc(cfg)
    in_maps = make_in_maps(cfg, pos_nx3, sdf_n, tet_fx4, unique_edges,
                           idx_map, n_verts, n_tets, n_edges, ngrid)
    res = run_bass_kernel_spmd(nc, in_maps,
                               core_ids=list(range(cfg.n_cores)))
    return assemble(cfg, res.results, n_verts, n_tets, n_edges, ngrid)
